# revision 6
# baseline (speedup 1.0000x reference)
"""Multi-head causal attention (B=4, T=2048, D=1024, H=16, HS=64) on 8 TRN2
NeuronCores.

Sharding: batch (4-way) x head-group (2-way).  Core c handles batch c//2 and
heads 8*(c%2) .. 8*(c%2)+7.  Each core computes its 8 heads' attention and the
full output projection Y_T = sum_h Wo_h^T @ O_T_h for its head group; the host
sums the two head-group partials per batch, transposes, and adds the bias.

Per-core program (matmuls contract along the partition dim; datapath bf16 with
fp32 PSUM, softmax denominator in fp32):
  - x^T arrives pre-transposed from the host in per-(t4,dc) DMA chunks so the
    first V-projection matmul can start ~2us in.
  - V^T/Q^T/K^T [e2, t] = matmul(lhsT=W[d, e2], rhs=x^T), head pairs packed on
    the PE M axis; V^T is re-transposed into V_aug [k, 65] (ones column -> the
    softmax denominator accumulates inside the attn@v matmul for free) via the
    DMA xbar transpose engine -- zero PE cycles.
  - S^T blocks [k=128, q<=512] = matmul(lhsT=K^T, rhs=Q^T) land in a single
    persistent 6-bank PSUM tile with rotating slot pairs: exp activations stay
    fused over 2 chunks and the S matmul has 3 m-steps of WAR slack (the
    scalar engine's exp stream is co-critical with the PE, ~39us vs ~43us per
    pair -- the slack absorbs its jitter).
  - exp on ScalarE (1/sqrt(HS) folded into the activation scale; no max
    subtraction -- |scores| <= ~6 so exp cannot overflow); causal mask =
    upper-tri 0/1 multiply on the diagonal sub-blocks + column offsets.
  - O^T_aug [65, q] accumulates over k chunks in PSUM; normalization: one
    vector copy out of PSUM, reciprocal on the [1,512] denominator row, gpsimd
    partition_broadcast, vector multiply.  No DRAM bounce.
  - Output projection Y^T[d,q] = sum_pairs matmul(lhsT=Wo[e2,d], rhs=O^T),
    accumulated across all 4 pairs directly in PSUM (no staging adds).

Engine-level scheduling: engines execute in order, so emission order is the
schedule.  S^T runs three m-steps ahead of attn@v.  Independent PE work is
spread EVENLY across the filler slots of each pair's attention (next pair's
Q/K projections for pairs 0-2; the output projection, lagging one q-chunk,
for pair 3 which iterates j-major) so the PE never idles while the scalar
engine catches up -- this also keeps the HAM clock gate at 2.4 GHz.  Only the
last q-chunk's 8 output-projection units (~7us, PE-dense) trail the final
attention group.
"""

import numpy as np

B, T, D = 4, 2048, 1024
H, HS = 16, 64
NCORES = 8
NPAIR = 4   # head pairs per core
ND = 8      # 128-wide d chunks
NT = 16     # 128-wide t chunks
NQ = 4      # 512-wide q chunks
NK = 16     # 128-wide k chunks
NSLOT = 6   # S^T PSUM slots (banks)

_CACHE = {}


def _build_program():
    import concourse.bass as bass
    import concourse.tile as tile
    from concourse import bacc, mybir
    from contextlib import ExitStack

    f32 = mybir.dt.float32
    bf16 = mybir.dt.bfloat16
    Exp = mybir.ActivationFunctionType.Exp

    nc = bacc.Bacc("TRN2", target_bir_lowering=False, debug=False)

    x_d = nc.declare_dram_parameter("x", [128, NQ, ND, 512], bf16, isOutput=False)
    wq_d = nc.declare_dram_parameter("wq", [NPAIR, 128, ND, 128], bf16, isOutput=False)
    wk_d = nc.declare_dram_parameter("wk", [NPAIR, 128, ND, 128], bf16, isOutput=False)
    wv_d = nc.declare_dram_parameter("wv", [NPAIR, 128, ND, 128], bf16, isOutput=False)
    wo_d = nc.declare_dram_parameter("wo", [128, NPAIR, ND, 128], bf16, isOutput=False)
    tri_d = nc.declare_dram_parameter("tri", [128, 128], bf16, isOutput=False)
    yt_d = nc.declare_dram_parameter("yt", [D, T], f32, isOutput=True)

    with tile.TileContext(nc) as tc, ExitStack() as top:
        const = top.enter_context(tc.tile_pool(name="const", bufs=1))
        tri_sb = const.tile([128, 128], bf16, name="tri_sb")
        nc.sync.dma_start(out=tri_sb, in_=tri_d[:, :])

        big = top.enter_context(tc.tile_pool(name="big", bufs=1))
        vaug = big.tile([128, 2 * NPAIR, NK, 65], bf16, name="vaug")
        nc.vector.memset(vaug[:, :, :, 64:65], 1.0)

        otn_p = top.enter_context(tc.tile_pool(name="otn_p", bufs=1))
        otn = otn_p.tile([128, NPAIR, T], bf16, name="otn")

        # PSUM banks: psM 1 + psS 6 + psO 1 = 8 (psA only lives in phase A)
        psM = top.enter_context(tc.tile_pool(name="psM", bufs=1, space="PSUM"))
        pw = top.enter_context(tc.tile_pool(name="pw", bufs=2))
        qkp = top.enter_context(tc.tile_pool(name="qkp", bufs=2))
        ptp = top.enter_context(tc.tile_pool(name="ptp", bufs=6))
        ocp = top.enter_context(tc.tile_pool(name="ocp", bufs=2))
        rcp = top.enter_context(tc.tile_pool(name="rcp", bufs=2))
        lbp = top.enter_context(tc.tile_pool(name="lbp", bufs=2))
        pyt = top.enter_context(tc.tile_pool(name="pyt", bufs=3))
        pwo = top.enter_context(tc.tile_pool(name="pwo", bufs=1))

        def dma_w(wdram, p, kind, pool=None):
            pool = pool or pw
            w_sb = pool.tile([128, ND, 128], bf16, tag="w", name=f"w_{kind}{p}")
            nc.sync.dma_start(out=w_sb, in_=wdram[p])
            return w_sb

        # ---- Phase A: x^T DMA / V-proj / xbar V-transpose / Q0/K0 ----------
        xtp = top.enter_context(tc.tile_pool(name="xtp", bufs=1))
        xt = xtp.tile([128, NQ, ND, 512], bf16, name="xt")
        with ExitStack() as ph:
            psA = ph.enter_context(tc.tile_pool(name="psA", bufs=4, space="PSUM"))
            vts = ph.enter_context(tc.tile_pool(name="vts", bufs=3))
            stg = ph.enter_context(tc.tile_pool(name="stg", bufs=4))
            pwv = ph.enter_context(tc.tile_pool(name="pwv", bufs=4))

            # DMA order: wv0 + x(t4=0) first so compute starts immediately.
            wv_sbs = [None] * NPAIR
            wv_sbs[0] = dma_w(wv_d, 0, "v", pool=pwv)
            for dc in range(ND):
                nc.sync.dma_start(out=xt[:, 0, dc, :], in_=x_d[:, 0, dc, :])
            for p in range(1, NPAIR):
                wv_sbs[p] = dma_w(wv_d, p, "v", pool=pwv)
            for t4 in range(1, NQ):
                for dc in range(ND):
                    nc.sync.dma_start(out=xt[:, t4, dc, :], in_=x_d[:, t4, dc, :])
            wq0 = dma_w(wq_d, 0, "q")
            wk0 = dma_w(wk_d, 0, "k")
            wo_sb = pwo.tile([128, NPAIR, ND, 128], bf16, name="wo_sb")
            nc.sync.dma_start(out=wo_sb, in_=wo_d[:, :, :, :])

            def proj_mms(ps_t4, w_sb, t4, dc_lo, dc_hi):
                for dc in range(dc_lo, dc_hi):
                    nc.tensor.matmul(
                        ps_t4,
                        w_sb[:, dc, :],
                        xt[:, t4, dc, :],
                        start=(dc == 0),
                        stop=(dc == ND - 1),
                    )

            vstash = {}

            def emit_vproj(pv):
                t4, p = pv // 4, pv % 4
                ps_t4 = psA.tile([128, 512], f32, tag="mm", name="psv")
                proj_mms(ps_t4, wv_sbs[p], t4, 0, ND)
                vt = vts.tile([128, 512], bf16, tag="vt", name="vt")
                nc.scalar.copy(out=vt, in_=ps_t4)
                vstash[pv] = vt

            def emit_vtr(pv):
                # V^T [e,t] block -> V_aug [k,e] via the DMA xbar transpose.
                # The xbar needs a 64B-aligned contiguous dest, so it lands in
                # a staging tile; one strided vector copy moves it into vaug.
                t4, p = pv // 4, pv % 4
                vt = vstash.pop(pv)
                for hh in range(2):
                    h = 2 * p + hh
                    st = stg.tile([128, 4, 64], bf16, tag="st", name="stg")
                    nc.sync.dma_start_transpose(
                        out=st[:, :, :],
                        in_=vt[hh * 64:hh * 64 + 64, :],
                    )
                    nc.vector.tensor_copy(
                        out=vaug[:, h, 4 * t4:4 * t4 + 4, 0:64], in_=st)

            for pv in range(4 * NPAIR):
                emit_vproj(pv)
                if pv >= 1:
                    emit_vtr(pv - 1)
            emit_vtr(4 * NPAIR - 1)

            qt0 = qkp.tile([128, T], bf16, tag="qt", name="qt0")
            kt0 = qkp.tile([128, T], bf16, tag="kt", name="kt0")
            for w_sb, dest, eng in ((wq0, qt0, "s"), (wk0, kt0, "v")):
                for t4 in range(NQ):
                    ps_t4 = psA.tile([128, 512], f32, tag="mm", name="psqk")
                    proj_mms(ps_t4, w_sb, t4, 0, ND)
                    dst = dest[:, t4 * 512:(t4 + 1) * 512]
                    if eng == "s":
                        nc.scalar.copy(out=dst, in_=ps_t4)
                    else:
                        nc.vector.tensor_copy(out=dst, in_=ps_t4)

        # ---- Attention machinery -------------------------------------------
        psSp = top.enter_context(tc.tile_pool(name="psS", bufs=1, space="PSUM"))
        psS = psSp.tile([128, NSLOT, 512], f32, name="psS")
        psO = top.enter_context(tc.tile_pool(name="psO", bufs=1, space="PSUM"))
        step_ctr = [0]  # global m-step counter -> psS slot-pair rotation

        def attn_group(p, hh, j, qt, kt, filler):
            """One (head, q-chunk) attention group with pipelined emission."""
            h = 2 * p + hh
            e0 = hh * 64
            po = psO.tile([65, 512], f32, tag="O", name="po")
            ncc = 4 * (j + 1)
            nm = ncc // 2
            pts = {}

            def off_of(c):
                sub = c - 4 * j
                return sub * 128 if 0 <= sub < 4 else 0

            def emit_s(m):
                s0 = (2 * step_ctr[0]) % NSLOT
                step_ctr[0] += 1
                pt = ptp.tile([128, 2, 512], bf16, tag="pt", name="pt")
                offs = []
                for i in range(2):
                    c = 2 * m + i
                    off = off_of(c)
                    offs.append(off)
                    nc.tensor.matmul(
                        psS[:, s0 + i, off:],
                        kt[e0:e0 + 64, c * 128:(c + 1) * 128],
                        qt[e0:e0 + 64, j * 512 + off:(j + 1) * 512],
                        start=True,
                        stop=True,
                    )
                if offs[0] == offs[1]:
                    nc.scalar.activation(out=pt[:, :, offs[0]:],
                                         in_=psS[:, s0:s0 + 2, offs[0]:],
                                         func=Exp, scale=0.125)
                else:
                    for i, off in enumerate(offs):
                        nc.scalar.activation(out=pt[:, i, off:],
                                             in_=psS[:, s0 + i, off:],
                                             func=Exp, scale=0.125)
                for i in range(2):
                    c = 2 * m + i
                    sub = c - 4 * j
                    if 0 <= sub < 4:
                        nc.vector.tensor_mul(
                            pt[:, i, sub * 128:(sub + 1) * 128],
                            pt[:, i, sub * 128:(sub + 1) * 128],
                            tri_sb,
                        )
                pts[m] = pt

            def emit_v(m):
                pt = pts.pop(m)
                for i in range(2):
                    c = 2 * m + i
                    off = off_of(c)
                    nc.tensor.matmul(
                        po[:, off:],
                        vaug[:, h, c, :],
                        pt[:, i, off:],
                        start=(c == 0),
                        stop=(c == ncc - 1),
                    )

            for m0 in range(min(3, nm)):
                emit_s(m0)
            for m in range(nm):
                if m + 3 < nm:
                    emit_s(m + 3)
                filler()
                emit_v(m)

            # normalize: otn[e, q] = O_T[e, q] / l[q]
            ocl = ocp.tile([64, 512], f32, tag="oc", name="ocl")
            nc.vector.tensor_copy(out=ocl, in_=po[0:64, :])
            rcl = rcp.tile([1, 512], f32, tag="rl", name="rcl")
            nc.vector.tensor_copy(out=rcl, in_=po[64:65, :])
            nc.vector.reciprocal_approx_fast(rcl, rcl)
            lbb = lbp.tile([64, 512], f32, tag="lb", name="lbb")
            nc.gpsimd.partition_broadcast(lbb, rcl)
            nc.vector.tensor_mul(
                otn[e0:e0 + 64, p, j * 512:(j + 1) * 512], ocl, lbb
            )

        class Spread:
            """Pop filler units evenly across a known number of slots."""

            def __init__(self, units, slots, start=0):
                self.units = list(units)
                self.n0 = len(self.units)
                self.slots = max(1, slots - start)
                self.start = start
                self.slot = 0
                self.popped = 0

            def __call__(self):
                self.slot += 1
                eff = max(0, self.slot - self.start)
                want = -(-self.n0 * eff // self.slots)  # ceil
                while self.popped < want and self.units:
                    self.units.pop(0)()
                    self.popped += 1

            def drain(self):
                while self.units:
                    self.units.pop(0)()

        # Q/K projection filler units (halves of 8-dc chains, shared state)
        def qk_units(p, qt_nxt, kt_nxt, wq_nxt, wk_nxt):
            units = []
            for w_sb, dest in ((wk_nxt, kt_nxt), (wq_nxt, qt_nxt)):
                for t4 in range(NQ):
                    state = {}

                    def mk(w_sb=w_sb, dest=dest, t4=t4, dc_lo=0, dc_hi=4,
                           state=state):
                        def emit():
                            if dc_lo == 0:
                                state["ps"] = psM.tile([128, 512], f32,
                                                       tag="mm", name="psf")
                            proj_mms(state["ps"], w_sb, t4, dc_lo, dc_hi)
                            if dc_hi == ND:
                                nc.vector.tensor_copy(
                                    out=dest[:, t4 * 512:(t4 + 1) * 512],
                                    in_=state["ps"])
                        return emit

                    units.append(mk(dc_lo=0, dc_hi=4))
                    units.append(mk(dc_lo=4, dc_hi=ND))
            # reorder: t4-major interleaved k/q so early chunks finish first
            order = []
            for t4 in range(NQ):
                for w in range(2):
                    order.append(units[w * 8 + 2 * t4])
                    order.append(units[w * 8 + 2 * t4 + 1])
            return order

        # ---- Phase B: pairs 0-2, next-pair Q/K spread as filler ------------
        qt_cur, kt_cur = qt0, kt0
        deferred = []
        for p in range(NPAIR - 1):
            qt_nxt = qkp.tile([128, T], bf16, tag="qt", name=f"qt{p+1}")
            kt_nxt = qkp.tile([128, T], bf16, tag="kt", name=f"kt{p+1}")
            wq_nxt = dma_w(wq_d, p + 1, "q")
            wk_nxt = dma_w(wk_d, p + 1, "k")
            units = qk_units(p, qt_nxt, kt_nxt, wq_nxt, wk_nxt)
            if p == NPAIR - 2:
                # defer pair-3's t4=3 halves into pair-3's j=0 slots
                deferred = units[12:16]
                units = units[:12]
            spread = Spread(units, 40)
            for hh in range(2):
                for j in range(NQ):
                    attn_group(p, hh, j, qt_cur, kt_cur, spread)
            spread.drain()
            qt_cur, kt_cur = qt_nxt, kt_nxt

        # ---- Pair 3: j-major attention + PSUM-accumulated out-projection --
        def oproj_unit(dc, qc):
            def emit():
                py = psM.tile([128, 512], f32, tag="mm", name="pyo")
                for pp in range(NPAIR):
                    nc.tensor.matmul(
                        py,
                        wo_sb[:, pp, dc, :],
                        otn[:, pp, qc * 512:(qc + 1) * 512],
                        start=(pp == 0),
                        stop=(pp == NPAIR - 1),
                    )
                yt_sb = pyt.tile([128, 512], f32, tag="yt", name="yt_f")
                nc.vector.tensor_copy(out=yt_sb, in_=py)
                nc.sync.dma_start(
                    out=yt_d[dc * 128:(dc + 1) * 128,
                             qc * 512:(qc + 1) * 512],
                    in_=yt_sb,
                )
            return emit

        for j in range(NQ):
            slots = 4 * (j + 1)  # both heads' filler slots at this j
            if j == 0:
                units = deferred
            else:
                units = [oproj_unit(dc, j - 1) for dc in range(ND)]
            spread = Spread(units, slots, start=1 if j else 0)
            attn_group(3, 0, j, qt_cur, kt_cur, spread)
            attn_group(3, 1, j, qt_cur, kt_cur, spread)
            spread.drain()
        for dc in range(ND):
            oproj_unit(dc, NQ - 1)()

    nc.compile()
    return nc


def _pack_inputs(x, Wq, Wk, Wv, Wo):
    """Per-core input maps. Core c: batch c//2, head group c%2."""
    import ml_dtypes

    tri = np.triu(np.ones((128, 128), np.float32)).astype(ml_dtypes.bfloat16)

    def pack_w(W, g):
        # [NPAIR, 128(d_local), ND, 128(e2)]
        out = np.empty((NPAIR, 128, ND, 128), np.float32)
        for p in range(NPAIR):
            h1 = 8 * g + 2 * p
            r = W[[h1, h1 + 1]].transpose(1, 0, 2).reshape(D, 128)  # [d, e2]
            out[p] = r.reshape(ND, 128, 128).transpose(1, 0, 2)
        return np.ascontiguousarray(out).astype(ml_dtypes.bfloat16)

    def pack_wo(Wo, g):
        # [128(e2), NPAIR, ND, 128(d)]
        out = np.empty((128, NPAIR, ND, 128), np.float32)
        for p in range(NPAIR):
            r0 = (8 * g + 2 * p) * 64
            out[:, p] = Wo[r0:r0 + 128].reshape(128, ND, 128)
        return np.ascontiguousarray(out).astype(ml_dtypes.bfloat16)

    packs = {}
    for g in range(2):
        packs[g] = dict(
            wq=pack_w(Wq, g), wk=pack_w(Wk, g), wv=pack_w(Wv, g),
            wo=pack_wo(Wo, g),
        )
    in_maps = []
    for c in range(NCORES):
        b, g = c // 2, c % 2
        m = dict(packs[g])
        xt = x[b].reshape(NQ, 512, ND, 128).transpose(3, 0, 2, 1)
        m["x"] = np.ascontiguousarray(xt).astype(ml_dtypes.bfloat16)
        m["tri"] = tri
        in_maps.append(m)
    return in_maps


def kernel(x, Wq, Wk, Wv, Wo, bo):
    from concourse.bass_utils import run_bass_kernel_spmd

    x = np.asarray(x, np.float32)
    Wq, Wk, Wv = (np.asarray(a, np.float32) for a in (Wq, Wk, Wv))
    Wo = np.asarray(Wo, np.float32)
    bo = np.asarray(bo, np.float32)

    if "nc" not in _CACHE:
        _CACHE["nc"] = _build_program()
    nc = _CACHE["nc"]

    in_maps = _pack_inputs(x, Wq, Wk, Wv, Wo)
    res = run_bass_kernel_spmd(nc, in_maps, list(range(NCORES)))
    _CACHE["last_result"] = res

    out = np.empty((B, T, D), np.float32)
    for b in range(B):
        yt = res.results[2 * b]["yt"] + res.results[2 * b + 1]["yt"]
        out[b] = yt.T + bo
    return out


# revision 7
# speedup vs baseline: 1.0130x; 1.0130x over previous
"""Multi-head causal attention (B=4, T=2048, D=1024, H=16, HS=64) on 8 TRN2
NeuronCores.

Sharding: batch (4-way) x head-group (2-way).  Core c handles batch c//2 and
heads 8*(c%2) .. 8*(c%2)+7.  Each core computes its 8 heads' attention and the
full output projection Y_T = sum_h Wo_h^T @ O_T_h for its head group; the host
sums the two head-group partials per batch, transposes, and adds the bias.

Per-core program (matmuls contract along the partition dim; datapath bf16 with
fp32 PSUM, softmax denominator in fp32):
  - x^T arrives pre-transposed from the host in per-(t4,dc) DMA chunks so the
    first V-projection matmul can start ~2us in.
  - V^T/Q^T/K^T [e2, t] = matmul(lhsT=W[d, e2], rhs=x^T), head pairs packed on
    the PE M axis; V^T is re-transposed into V_aug [k, 65] (ones column -> the
    softmax denominator accumulates inside the attn@v matmul for free) via the
    DMA xbar transpose engine -- zero PE cycles.
  - S^T blocks [k=128, q<=512] = matmul(lhsT=K^T, rhs=Q^T) land in a single
    persistent 6-bank PSUM tile with rotating slot pairs: exp activations stay
    fused over 2 chunks and the S matmul has 3 m-steps of WAR slack (the
    scalar engine's exp stream is co-critical with the PE, ~39us vs ~43us per
    pair -- the slack absorbs its jitter).
  - exp on ScalarE (1/sqrt(HS) folded into the activation scale; no max
    subtraction -- |scores| <= ~6 so exp cannot overflow); causal mask =
    upper-tri 0/1 multiply on the diagonal sub-blocks + column offsets.
  - O^T_aug [65, q] accumulates over k chunks in PSUM; normalization: one
    vector copy out of PSUM, reciprocal on the [1,512] denominator row, gpsimd
    partition_broadcast, vector multiply.  No DRAM bounce.
  - Output projection Y^T[d,q] = sum_pairs matmul(lhsT=Wo[e2,d], rhs=O^T),
    accumulated across all 4 pairs directly in PSUM (no staging adds).

Engine-level scheduling: engines execute in order, so emission order is the
schedule.  S^T runs three m-steps ahead of attn@v.  Independent PE work is
spread EVENLY across the filler slots of each pair's attention (next pair's
Q/K projections for pairs 0-2; the output projection, lagging one q-chunk,
for pair 3 which iterates j-major) so the PE never idles while the scalar
engine catches up -- this also keeps the HAM clock gate at 2.4 GHz.  Only the
last q-chunk's 8 output-projection units (~7us, PE-dense) trail the final
attention group.
"""

import numpy as np

B, T, D = 4, 2048, 1024
H, HS = 16, 64
NCORES = 8
NPAIR = 4   # head pairs per core
ND = 8      # 128-wide d chunks
NT = 16     # 128-wide t chunks
NQ = 4      # 512-wide q chunks
NK = 16     # 128-wide k chunks
NSLOT = 6   # S^T PSUM slots (banks)

_CACHE = {}


def _build_program():
    import concourse.bass as bass
    import concourse.tile as tile
    from concourse import bacc, mybir
    from contextlib import ExitStack

    f32 = mybir.dt.float32
    bf16 = mybir.dt.bfloat16
    Exp = mybir.ActivationFunctionType.Exp

    nc = bacc.Bacc("TRN2", target_bir_lowering=False, debug=False)

    x_d = nc.declare_dram_parameter("x", [128, NQ, ND, 512], bf16, isOutput=False)
    wq_d = nc.declare_dram_parameter("wq", [NPAIR, 128, ND, 128], bf16, isOutput=False)
    wk_d = nc.declare_dram_parameter("wk", [NPAIR, 128, ND, 128], bf16, isOutput=False)
    wv_d = nc.declare_dram_parameter("wv", [NPAIR, 128, ND, 128], bf16, isOutput=False)
    wo_d = nc.declare_dram_parameter("wo", [128, NPAIR, ND, 128], bf16, isOutput=False)
    tri_d = nc.declare_dram_parameter("tri", [128, 128], bf16, isOutput=False)
    yt_d = nc.declare_dram_parameter("yt", [D, T], f32, isOutput=True)

    with tile.TileContext(nc) as tc, ExitStack() as top:
        const = top.enter_context(tc.tile_pool(name="const", bufs=1))
        tri_sb = const.tile([128, 128], bf16, name="tri_sb")
        nc.sync.dma_start(out=tri_sb, in_=tri_d[:, :])

        big = top.enter_context(tc.tile_pool(name="big", bufs=1))
        vaug = big.tile([128, 2 * NPAIR, NK, 65], bf16, name="vaug")
        nc.vector.memset(vaug[:, :, :, 64:65], 1.0)

        otn_p = top.enter_context(tc.tile_pool(name="otn_p", bufs=1))
        otn = otn_p.tile([128, NPAIR, T], bf16, name="otn")

        # PSUM banks: psM 1 + psS 6 + psO 1 = 8 (psA only lives in phase A)
        psM = top.enter_context(tc.tile_pool(name="psM", bufs=1, space="PSUM"))
        pw = top.enter_context(tc.tile_pool(name="pw", bufs=2))
        qkp = top.enter_context(tc.tile_pool(name="qkp", bufs=2))
        ptp = top.enter_context(tc.tile_pool(name="ptp", bufs=6))
        ocp = top.enter_context(tc.tile_pool(name="ocp", bufs=2))
        rcp = top.enter_context(tc.tile_pool(name="rcp", bufs=2))
        lbp = top.enter_context(tc.tile_pool(name="lbp", bufs=2))
        pyt = top.enter_context(tc.tile_pool(name="pyt", bufs=3))
        pwo = top.enter_context(tc.tile_pool(name="pwo", bufs=1))

        def dma_w(wdram, p, kind, pool=None):
            pool = pool or pw
            w_sb = pool.tile([128, ND, 128], bf16, tag="w", name=f"w_{kind}{p}")
            nc.sync.dma_start(out=w_sb, in_=wdram[p])
            return w_sb

        # ---- Phase A: x^T DMA / V-proj / xbar V-transpose / Q0/K0 ----------
        xtp = top.enter_context(tc.tile_pool(name="xtp", bufs=1))
        xt = xtp.tile([128, NQ, ND, 512], bf16, name="xt")
        with ExitStack() as ph:
            psA = ph.enter_context(tc.tile_pool(name="psA", bufs=4, space="PSUM"))
            vts = ph.enter_context(tc.tile_pool(name="vts", bufs=3))
            stg = ph.enter_context(tc.tile_pool(name="stg", bufs=4))
            pwv = ph.enter_context(tc.tile_pool(name="pwv", bufs=4))

            # DMA order: wv0 + x(t4=0) first so compute starts immediately.
            # t4=0 arrives per-dc so the first accumulation chain can chase the
            # DMA; later chunks use big 8KB-per-partition transfers for full
            # HBM efficiency (small-line DMAs halve effective bandwidth).
            wv_sbs = [None] * NPAIR
            wv_sbs[0] = dma_w(wv_d, 0, "v", pool=pwv)
            for dc in range(ND):
                nc.sync.dma_start(out=xt[:, 0, dc, :], in_=x_d[:, 0, dc, :])
            for p in range(1, NPAIR):
                wv_sbs[p] = dma_w(wv_d, p, "v", pool=pwv)
            nc.sync.dma_start(out=xt[:, 1, 0:4, :], in_=x_d[:, 1, 0:4, :])
            nc.sync.dma_start(out=xt[:, 1, 4:8, :], in_=x_d[:, 1, 4:8, :])
            for t4 in range(2, NQ):
                nc.sync.dma_start(out=xt[:, t4, :, :], in_=x_d[:, t4, :, :])
            wq0 = dma_w(wq_d, 0, "q")
            wk0 = dma_w(wk_d, 0, "k")
            wo_sb = pwo.tile([128, NPAIR, ND, 128], bf16, name="wo_sb")
            nc.sync.dma_start(out=wo_sb, in_=wo_d[:, :, :, :])

            def proj_mms(ps_t4, w_sb, t4, dc_lo, dc_hi):
                for dc in range(dc_lo, dc_hi):
                    nc.tensor.matmul(
                        ps_t4,
                        w_sb[:, dc, :],
                        xt[:, t4, dc, :],
                        start=(dc == 0),
                        stop=(dc == ND - 1),
                    )

            vstash = {}

            def emit_vproj(pv):
                t4, p = pv // 4, pv % 4
                ps_t4 = psA.tile([128, 512], f32, tag="mm", name="psv")
                proj_mms(ps_t4, wv_sbs[p], t4, 0, ND)
                vt = vts.tile([128, 512], bf16, tag="vt", name="vt")
                nc.scalar.copy(out=vt, in_=ps_t4)
                vstash[pv] = vt

            def emit_vtr(pv):
                # V^T [e,t] block -> V_aug [k,e] via the DMA xbar transpose.
                # The xbar needs a 64B-aligned contiguous dest, so it lands in
                # a staging tile; one strided vector copy moves it into vaug.
                t4, p = pv // 4, pv % 4
                vt = vstash.pop(pv)
                for hh in range(2):
                    h = 2 * p + hh
                    st = stg.tile([128, 4, 64], bf16, tag="st", name="stg")
                    nc.sync.dma_start_transpose(
                        out=st[:, :, :],
                        in_=vt[hh * 64:hh * 64 + 64, :],
                    )
                    nc.vector.tensor_copy(
                        out=vaug[:, h, 4 * t4:4 * t4 + 4, 0:64], in_=st)

            for pv in range(4 * NPAIR):
                emit_vproj(pv)
                if pv >= 1:
                    emit_vtr(pv - 1)
            emit_vtr(4 * NPAIR - 1)

            qt0 = qkp.tile([128, T], bf16, tag="qt", name="qt0")
            kt0 = qkp.tile([128, T], bf16, tag="kt", name="kt0")
            for w_sb, dest, eng in ((wq0, qt0, "s"), (wk0, kt0, "v")):
                for t4 in range(NQ):
                    ps_t4 = psA.tile([128, 512], f32, tag="mm", name="psqk")
                    proj_mms(ps_t4, w_sb, t4, 0, ND)
                    dst = dest[:, t4 * 512:(t4 + 1) * 512]
                    if eng == "s":
                        nc.scalar.copy(out=dst, in_=ps_t4)
                    else:
                        nc.vector.tensor_copy(out=dst, in_=ps_t4)

        # ---- Attention machinery -------------------------------------------
        psSp = top.enter_context(tc.tile_pool(name="psS", bufs=1, space="PSUM"))
        psS = psSp.tile([128, NSLOT, 512], f32, name="psS")
        psO = top.enter_context(tc.tile_pool(name="psO", bufs=1, space="PSUM"))
        step_ctr = [0]  # global m-step counter -> psS slot-pair rotation

        def attn_group(p, hh, j, qt, kt, filler):
            """One (head, q-chunk) attention group with pipelined emission."""
            h = 2 * p + hh
            e0 = hh * 64
            po = psO.tile([65, 512], f32, tag="O", name="po")
            ncc = 4 * (j + 1)
            nm = ncc // 2
            pts = {}

            def off_of(c):
                sub = c - 4 * j
                return sub * 128 if 0 <= sub < 4 else 0

            def emit_s(m):
                s0 = (2 * step_ctr[0]) % NSLOT
                step_ctr[0] += 1
                pt = ptp.tile([128, 2, 512], bf16, tag="pt", name="pt")
                offs = []
                for i in range(2):
                    c = 2 * m + i
                    off = off_of(c)
                    offs.append(off)
                    nc.tensor.matmul(
                        psS[:, s0 + i, off:],
                        kt[e0:e0 + 64, c * 128:(c + 1) * 128],
                        qt[e0:e0 + 64, j * 512 + off:(j + 1) * 512],
                        start=True,
                        stop=True,
                    )
                if offs[0] == offs[1]:
                    nc.scalar.activation(out=pt[:, :, offs[0]:],
                                         in_=psS[:, s0:s0 + 2, offs[0]:],
                                         func=Exp, scale=0.125)
                else:
                    for i, off in enumerate(offs):
                        nc.scalar.activation(out=pt[:, i, off:],
                                             in_=psS[:, s0 + i, off:],
                                             func=Exp, scale=0.125)
                for i in range(2):
                    c = 2 * m + i
                    sub = c - 4 * j
                    if 0 <= sub < 4:
                        nc.vector.tensor_mul(
                            pt[:, i, sub * 128:(sub + 1) * 128],
                            pt[:, i, sub * 128:(sub + 1) * 128],
                            tri_sb,
                        )
                pts[m] = pt

            def emit_v(m):
                pt = pts.pop(m)
                for i in range(2):
                    c = 2 * m + i
                    off = off_of(c)
                    nc.tensor.matmul(
                        po[:, off:],
                        vaug[:, h, c, :],
                        pt[:, i, off:],
                        start=(c == 0),
                        stop=(c == ncc - 1),
                    )

            for m0 in range(min(3, nm)):
                emit_s(m0)
            for m in range(nm):
                if m + 3 < nm:
                    emit_s(m + 3)
                filler()
                emit_v(m)

            # normalize: otn[e, q] = O_T[e, q] / l[q]
            ocl = ocp.tile([64, 512], f32, tag="oc", name="ocl")
            nc.vector.tensor_copy(out=ocl, in_=po[0:64, :])
            rcl = rcp.tile([1, 512], f32, tag="rl", name="rcl")
            nc.vector.tensor_copy(out=rcl, in_=po[64:65, :])
            nc.vector.reciprocal_approx_fast(rcl, rcl)
            lbb = lbp.tile([64, 512], f32, tag="lb", name="lbb")
            nc.gpsimd.partition_broadcast(lbb, rcl)
            nc.vector.tensor_mul(
                otn[e0:e0 + 64, p, j * 512:(j + 1) * 512], ocl, lbb
            )

        class Spread:
            """Pop filler units evenly across a known number of slots."""

            def __init__(self, units, slots, start=0):
                self.units = list(units)
                self.n0 = len(self.units)
                self.slots = max(1, slots - start)
                self.start = start
                self.slot = 0
                self.popped = 0

            def __call__(self):
                self.slot += 1
                eff = max(0, self.slot - self.start)
                want = -(-self.n0 * eff // self.slots)  # ceil
                while self.popped < want and self.units:
                    self.units.pop(0)()
                    self.popped += 1

            def drain(self):
                while self.units:
                    self.units.pop(0)()

        # Q/K projection filler units (halves of 8-dc chains, shared state)
        def qk_units(p, qt_nxt, kt_nxt, wq_nxt, wk_nxt):
            units = []
            for w_sb, dest in ((wk_nxt, kt_nxt), (wq_nxt, qt_nxt)):
                for t4 in range(NQ):
                    state = {}

                    def mk(w_sb=w_sb, dest=dest, t4=t4, dc_lo=0, dc_hi=4,
                           state=state):
                        def emit():
                            if dc_lo == 0:
                                state["ps"] = psM.tile([128, 512], f32,
                                                       tag="mm", name="psf")
                            proj_mms(state["ps"], w_sb, t4, dc_lo, dc_hi)
                            if dc_hi == ND:
                                nc.vector.tensor_copy(
                                    out=dest[:, t4 * 512:(t4 + 1) * 512],
                                    in_=state["ps"])
                        return emit

                    units.append(mk(dc_lo=0, dc_hi=4))
                    units.append(mk(dc_lo=4, dc_hi=ND))
            # reorder: t4-major interleaved k/q so early chunks finish first
            order = []
            for t4 in range(NQ):
                for w in range(2):
                    order.append(units[w * 8 + 2 * t4])
                    order.append(units[w * 8 + 2 * t4 + 1])
            return order

        # ---- Phase B: pairs 0-2, next-pair Q/K spread as filler ------------
        qt_cur, kt_cur = qt0, kt0
        deferred = []
        for p in range(NPAIR - 1):
            qt_nxt = qkp.tile([128, T], bf16, tag="qt", name=f"qt{p+1}")
            kt_nxt = qkp.tile([128, T], bf16, tag="kt", name=f"kt{p+1}")
            wq_nxt = dma_w(wq_d, p + 1, "q")
            wk_nxt = dma_w(wk_d, p + 1, "k")
            units = qk_units(p, qt_nxt, kt_nxt, wq_nxt, wk_nxt)
            if p == NPAIR - 2:
                # defer pair-3's t4=3 halves into pair-3's j=0 slots
                deferred = units[12:16]
                units = units[:12]
            spread = Spread(units, 40)
            for hh in range(2):
                for j in range(NQ):
                    attn_group(p, hh, j, qt_cur, kt_cur, spread)
            spread.drain()
            qt_cur, kt_cur = qt_nxt, kt_nxt

        # ---- Pair 3: j-major attention + PSUM-accumulated out-projection --
        def oproj_unit(dc, qc):
            def emit():
                py = psM.tile([128, 512], f32, tag="mm", name="pyo")
                for pp in range(NPAIR):
                    nc.tensor.matmul(
                        py,
                        wo_sb[:, pp, dc, :],
                        otn[:, pp, qc * 512:(qc + 1) * 512],
                        start=(pp == 0),
                        stop=(pp == NPAIR - 1),
                    )
                yt_sb = pyt.tile([128, 512], f32, tag="yt", name="yt_f")
                nc.vector.tensor_copy(out=yt_sb, in_=py)
                nc.sync.dma_start(
                    out=yt_d[dc * 128:(dc + 1) * 128,
                             qc * 512:(qc + 1) * 512],
                    in_=yt_sb,
                )
            return emit

        for j in range(NQ):
            slots = 4 * (j + 1)  # both heads' filler slots at this j
            if j == 0:
                units = deferred
            else:
                units = [oproj_unit(dc, j - 1) for dc in range(ND)]
            spread = Spread(units, slots, start=1 if j else 0)
            attn_group(3, 0, j, qt_cur, kt_cur, spread)
            attn_group(3, 1, j, qt_cur, kt_cur, spread)
            spread.drain()
        for dc in range(ND):
            oproj_unit(dc, NQ - 1)()

    nc.compile()
    return nc


def _pack_inputs(x, Wq, Wk, Wv, Wo):
    """Per-core input maps. Core c: batch c//2, head group c%2."""
    import ml_dtypes

    tri = np.triu(np.ones((128, 128), np.float32)).astype(ml_dtypes.bfloat16)

    def pack_w(W, g):
        # [NPAIR, 128(d_local), ND, 128(e2)]
        out = np.empty((NPAIR, 128, ND, 128), np.float32)
        for p in range(NPAIR):
            h1 = 8 * g + 2 * p
            r = W[[h1, h1 + 1]].transpose(1, 0, 2).reshape(D, 128)  # [d, e2]
            out[p] = r.reshape(ND, 128, 128).transpose(1, 0, 2)
        return np.ascontiguousarray(out).astype(ml_dtypes.bfloat16)

    def pack_wo(Wo, g):
        # [128(e2), NPAIR, ND, 128(d)]
        out = np.empty((128, NPAIR, ND, 128), np.float32)
        for p in range(NPAIR):
            r0 = (8 * g + 2 * p) * 64
            out[:, p] = Wo[r0:r0 + 128].reshape(128, ND, 128)
        return np.ascontiguousarray(out).astype(ml_dtypes.bfloat16)

    packs = {}
    for g in range(2):
        packs[g] = dict(
            wq=pack_w(Wq, g), wk=pack_w(Wk, g), wv=pack_w(Wv, g),
            wo=pack_wo(Wo, g),
        )
    in_maps = []
    for c in range(NCORES):
        b, g = c // 2, c % 2
        m = dict(packs[g])
        xt = x[b].reshape(NQ, 512, ND, 128).transpose(3, 0, 2, 1)
        m["x"] = np.ascontiguousarray(xt).astype(ml_dtypes.bfloat16)
        m["tri"] = tri
        in_maps.append(m)
    return in_maps


def kernel(x, Wq, Wk, Wv, Wo, bo):
    from concourse.bass_utils import run_bass_kernel_spmd

    x = np.asarray(x, np.float32)
    Wq, Wk, Wv = (np.asarray(a, np.float32) for a in (Wq, Wk, Wv))
    Wo = np.asarray(Wo, np.float32)
    bo = np.asarray(bo, np.float32)

    if "nc" not in _CACHE:
        _CACHE["nc"] = _build_program()
    nc = _CACHE["nc"]

    in_maps = _pack_inputs(x, Wq, Wk, Wv, Wo)
    res = run_bass_kernel_spmd(nc, in_maps, list(range(NCORES)))
    _CACHE["last_result"] = res

    out = np.empty((B, T, D), np.float32)
    for b in range(B):
        yt = res.results[2 * b]["yt"] + res.results[2 * b + 1]["yt"]
        out[b] = yt.T + bo
    return out


# revision 8
# speedup vs baseline: 1.0211x; 1.0079x over previous
"""Multi-head causal attention (B=4, T=2048, D=1024, H=16, HS=64) on 8 TRN2
NeuronCores.

Sharding: batch (4-way) x head-group (2-way).  Core c handles batch c//2 and
heads 8*(c%2) .. 8*(c%2)+7.  Each core computes its 8 heads' attention and the
full output projection Y_T = sum_h Wo_h^T @ O_T_h for its head group; the host
sums the two head-group partials per batch, transposes, and adds the bias.

Per-core program (matmuls contract along the partition dim; datapath bf16 with
fp32 PSUM, softmax denominator in fp32):
  - x^T arrives pre-transposed from the host in per-(t4,dc) DMA chunks so the
    first V-projection matmul can start ~2us in.
  - V^T/Q^T/K^T [e2, t] = matmul(lhsT=W[d, e2], rhs=x^T), head pairs packed on
    the PE M axis; V^T is re-transposed into V_aug [k, 65] (ones column -> the
    softmax denominator accumulates inside the attn@v matmul for free) via the
    DMA xbar transpose engine -- zero PE cycles.
  - S^T blocks [k=128, q<=512] = matmul(lhsT=K^T, rhs=Q^T) land in a single
    persistent 6-bank PSUM tile with rotating slot pairs: exp activations stay
    fused over 2 chunks and the S matmul has 3 m-steps of WAR slack (the
    scalar engine's exp stream is co-critical with the PE, ~39us vs ~43us per
    pair -- the slack absorbs its jitter).
  - exp on ScalarE (1/sqrt(HS) folded into the activation scale; no max
    subtraction -- |scores| <= ~6 so exp cannot overflow); causal mask =
    upper-tri 0/1 multiply on the diagonal sub-blocks + column offsets.
  - O^T_aug [65, q] accumulates over k chunks in PSUM; normalization: one
    vector copy out of PSUM, reciprocal on the [1,512] denominator row, gpsimd
    partition_broadcast, vector multiply.  No DRAM bounce.
  - Output projection Y^T[d,q] = sum_pairs matmul(lhsT=Wo[e2,d], rhs=O^T),
    accumulated across all 4 pairs directly in PSUM (no staging adds).

Engine-level scheduling: engines execute in order, so emission order is the
schedule.  S^T runs three m-steps ahead of attn@v.  Independent PE work is
spread EVENLY across the filler slots of each pair's attention (next pair's
Q/K projections for pairs 0-2; the output projection, lagging one q-chunk,
for pair 3 which iterates j-major) so the PE never idles while the scalar
engine catches up -- this also keeps the HAM clock gate at 2.4 GHz.  Only the
last q-chunk's 8 output-projection units (~7us, PE-dense) trail the final
attention group.
"""

import numpy as np

B, T, D = 4, 2048, 1024
H, HS = 16, 64
NCORES = 8
NPAIR = 4   # head pairs per core
ND = 8      # 128-wide d chunks
NT = 16     # 128-wide t chunks
NQ = 4      # 512-wide q chunks
NK = 16     # 128-wide k chunks
NSLOT = 4   # S^T PSUM slots (banks)

_CACHE = {}


def _build_program():
    import concourse.bass as bass
    import concourse.tile as tile
    from concourse import bacc, mybir
    from contextlib import ExitStack

    f32 = mybir.dt.float32
    bf16 = mybir.dt.bfloat16
    Exp = mybir.ActivationFunctionType.Exp

    nc = bacc.Bacc("TRN2", target_bir_lowering=False, debug=False)

    x_d = nc.declare_dram_parameter("x", [128, NQ, ND, 512], bf16, isOutput=False)
    wq_d = nc.declare_dram_parameter("wq", [NPAIR, 128, ND, 128], bf16, isOutput=False)
    wk_d = nc.declare_dram_parameter("wk", [NPAIR, 128, ND, 128], bf16, isOutput=False)
    wv_d = nc.declare_dram_parameter("wv", [NPAIR, 128, ND, 128], bf16, isOutput=False)
    wo_d = nc.declare_dram_parameter("wo", [128, NPAIR, ND, 128], bf16, isOutput=False)
    tri_d = nc.declare_dram_parameter("tri", [128, 128], bf16, isOutput=False)
    yt_d = nc.declare_dram_parameter("yt", [D, T], f32, isOutput=True)

    with tile.TileContext(nc) as tc, ExitStack() as top:
        const = top.enter_context(tc.tile_pool(name="const", bufs=1))
        tri_sb = const.tile([128, 128], bf16, name="tri_sb")
        nc.sync.dma_start(out=tri_sb, in_=tri_d[:, :])

        big = top.enter_context(tc.tile_pool(name="big", bufs=1))
        vaug = big.tile([128, 2 * NPAIR, NK, 65], bf16, name="vaug")
        nc.vector.memset(vaug[:, :, :, 64:65], 1.0)

        otn_p = top.enter_context(tc.tile_pool(name="otn_p", bufs=1))
        otn = otn_p.tile([128, NPAIR, T], bf16, name="otn")

        # PSUM banks: psM 2 + psS 4 + psO 2 = 8 (psA only lives in phase A)
        psM = top.enter_context(tc.tile_pool(name="psM", bufs=2, space="PSUM"))
        pw = top.enter_context(tc.tile_pool(name="pw", bufs=2))
        qkp = top.enter_context(tc.tile_pool(name="qkp", bufs=2))
        ptp = top.enter_context(tc.tile_pool(name="ptp", bufs=8))
        ocp = top.enter_context(tc.tile_pool(name="ocp", bufs=2))
        rcp = top.enter_context(tc.tile_pool(name="rcp", bufs=2))
        lbp = top.enter_context(tc.tile_pool(name="lbp", bufs=2))
        pyt = top.enter_context(tc.tile_pool(name="pyt", bufs=3))
        pwo = top.enter_context(tc.tile_pool(name="pwo", bufs=1))

        def dma_w(wdram, p, kind, pool=None):
            pool = pool or pw
            w_sb = pool.tile([128, ND, 128], bf16, tag="w", name=f"w_{kind}{p}")
            nc.sync.dma_start(out=w_sb, in_=wdram[p])
            return w_sb

        # ---- Phase A: x^T DMA / V-proj / xbar V-transpose / Q0/K0 ----------
        xtp = top.enter_context(tc.tile_pool(name="xtp", bufs=1))
        xt = xtp.tile([128, NQ, ND, 512], bf16, name="xt")
        with ExitStack() as ph:
            psA = ph.enter_context(tc.tile_pool(name="psA", bufs=4, space="PSUM"))
            vts = ph.enter_context(tc.tile_pool(name="vts", bufs=3))
            stg = ph.enter_context(tc.tile_pool(name="stg", bufs=4))
            pwv = ph.enter_context(tc.tile_pool(name="pwv", bufs=4))

            # DMA order: wv0 + x(t4=0) first so compute starts immediately.
            # t4=0 arrives per-dc so the first accumulation chain can chase the
            # DMA; later chunks use big 8KB-per-partition transfers for full
            # HBM efficiency (small-line DMAs halve effective bandwidth).
            wv_sbs = [None] * NPAIR
            wv_sbs[0] = dma_w(wv_d, 0, "v", pool=pwv)
            for dc in range(ND):
                nc.sync.dma_start(out=xt[:, 0, dc, :], in_=x_d[:, 0, dc, :])
            for p in range(1, NPAIR):
                wv_sbs[p] = dma_w(wv_d, p, "v", pool=pwv)
            nc.sync.dma_start(out=xt[:, 1, 0:4, :], in_=x_d[:, 1, 0:4, :])
            nc.sync.dma_start(out=xt[:, 1, 4:8, :], in_=x_d[:, 1, 4:8, :])
            for t4 in range(2, NQ):
                nc.sync.dma_start(out=xt[:, t4, :, :], in_=x_d[:, t4, :, :])
            wq0 = dma_w(wq_d, 0, "q")
            wk0 = dma_w(wk_d, 0, "k")
            wo_sb = pwo.tile([128, NPAIR, ND, 128], bf16, name="wo_sb")
            nc.sync.dma_start(out=wo_sb, in_=wo_d[:, :, :, :])

            def proj_mms(ps_t4, w_sb, t4, dc_lo, dc_hi):
                for dc in range(dc_lo, dc_hi):
                    nc.tensor.matmul(
                        ps_t4,
                        w_sb[:, dc, :],
                        xt[:, t4, dc, :],
                        start=(dc == 0),
                        stop=(dc == ND - 1),
                    )

            vstash = {}

            def emit_vproj(pv):
                t4, p = pv // 4, pv % 4
                ps_t4 = psA.tile([128, 512], f32, tag="mm", name="psv")
                proj_mms(ps_t4, wv_sbs[p], t4, 0, ND)
                vt = vts.tile([128, 512], bf16, tag="vt", name="vt")
                nc.scalar.copy(out=vt, in_=ps_t4)
                vstash[pv] = vt

            def emit_vtr(pv):
                # V^T [e,t] block -> V_aug [k,e] via the DMA xbar transpose.
                # The xbar needs a 64B-aligned contiguous dest, so it lands in
                # a staging tile; one strided vector copy moves it into vaug.
                t4, p = pv // 4, pv % 4
                vt = vstash.pop(pv)
                for hh in range(2):
                    h = 2 * p + hh
                    st = stg.tile([128, 4, 64], bf16, tag="st", name="stg")
                    nc.sync.dma_start_transpose(
                        out=st[:, :, :],
                        in_=vt[hh * 64:hh * 64 + 64, :],
                    )
                    nc.vector.tensor_copy(
                        out=vaug[:, h, 4 * t4:4 * t4 + 4, 0:64], in_=st)

            for pv in range(4 * NPAIR):
                emit_vproj(pv)
                if pv >= 1:
                    emit_vtr(pv - 1)
            emit_vtr(4 * NPAIR - 1)

            qt0 = qkp.tile([128, T], bf16, tag="qt", name="qt0")
            kt0 = qkp.tile([128, T], bf16, tag="kt", name="kt0")
            for w_sb, dest, eng in ((wq0, qt0, "s"), (wk0, kt0, "v")):
                for t4 in range(NQ):
                    ps_t4 = psA.tile([128, 512], f32, tag="mm", name="psqk")
                    proj_mms(ps_t4, w_sb, t4, 0, ND)
                    dst = dest[:, t4 * 512:(t4 + 1) * 512]
                    if eng == "s":
                        nc.scalar.copy(out=dst, in_=ps_t4)
                    else:
                        nc.vector.tensor_copy(out=dst, in_=ps_t4)

        # ---- Attention machinery -------------------------------------------
        psSp = top.enter_context(tc.tile_pool(name="psS", bufs=1, space="PSUM"))
        psS = psSp.tile([128, NSLOT, 512], f32, name="psS")
        psO = top.enter_context(tc.tile_pool(name="psO", bufs=2, space="PSUM"))
        step_ctr = [0]  # global m-step counter -> psS slot-pair rotation

        def attn_group(p, hh, j, qt, kt, filler):
            """One (head, q-chunk) attention group with pipelined emission."""
            h = 2 * p + hh
            e0 = hh * 64
            po = psO.tile([65, 512], f32, tag="O", name="po")
            ncc = 4 * (j + 1)
            nm = ncc // 2
            pts = {}

            def off_of(c):
                sub = c - 4 * j
                return sub * 128 if 0 <= sub < 4 else 0

            def emit_s(m):
                s0 = (2 * step_ctr[0]) % NSLOT
                step_ctr[0] += 1
                pt = ptp.tile([128, 2, 512], bf16, tag="pt", name="pt")
                offs = []
                for i in range(2):
                    c = 2 * m + i
                    off = off_of(c)
                    offs.append(off)
                    nc.tensor.matmul(
                        psS[:, s0 + i, off:],
                        kt[e0:e0 + 64, c * 128:(c + 1) * 128],
                        qt[e0:e0 + 64, j * 512 + off:(j + 1) * 512],
                        start=True,
                        stop=True,
                    )
                if offs[0] == offs[1]:
                    nc.scalar.activation(out=pt[:, :, offs[0]:],
                                         in_=psS[:, s0:s0 + 2, offs[0]:],
                                         func=Exp, scale=0.125)
                else:
                    for i, off in enumerate(offs):
                        nc.scalar.activation(out=pt[:, i, off:],
                                             in_=psS[:, s0 + i, off:],
                                             func=Exp, scale=0.125)
                for i in range(2):
                    c = 2 * m + i
                    sub = c - 4 * j
                    if 0 <= sub < 4:
                        nc.vector.tensor_mul(
                            pt[:, i, sub * 128:(sub + 1) * 128],
                            pt[:, i, sub * 128:(sub + 1) * 128],
                            tri_sb,
                        )
                pts[m] = pt

            def emit_v(m):
                pt = pts.pop(m)
                for i in range(2):
                    c = 2 * m + i
                    off = off_of(c)
                    nc.tensor.matmul(
                        po[:, off:],
                        vaug[:, h, c, :],
                        pt[:, i, off:],
                        start=(c == 0),
                        stop=(c == ncc - 1),
                    )

            for m0 in range(min(2, nm)):
                emit_s(m0)
            for m in range(nm):
                if m + 2 < nm:
                    emit_s(m + 2)
                filler()
                emit_v(m)

            # normalize: otn[e, q] = O_T[e, q] / l[q]
            ocl = ocp.tile([64, 512], f32, tag="oc", name="ocl")
            nc.vector.tensor_copy(out=ocl, in_=po[0:64, :])
            rcl = rcp.tile([1, 512], f32, tag="rl", name="rcl")
            nc.vector.tensor_copy(out=rcl, in_=po[64:65, :])
            nc.vector.reciprocal_approx_fast(rcl, rcl)
            lbb = lbp.tile([64, 512], f32, tag="lb", name="lbb")
            nc.gpsimd.partition_broadcast(lbb, rcl)
            nc.vector.tensor_mul(
                otn[e0:e0 + 64, p, j * 512:(j + 1) * 512], ocl, lbb
            )

        class Spread:
            """Pop filler units evenly across a known number of slots."""

            def __init__(self, units, slots, start=0):
                self.units = list(units)
                self.n0 = len(self.units)
                self.slots = max(1, slots - start)
                self.start = start
                self.slot = 0
                self.popped = 0

            def __call__(self):
                self.slot += 1
                eff = max(0, self.slot - self.start)
                want = -(-self.n0 * eff // self.slots)  # ceil
                while self.popped < want and self.units:
                    self.units.pop(0)()
                    self.popped += 1

            def drain(self):
                while self.units:
                    self.units.pop(0)()

        # Q/K projection filler units (halves of 8-dc chains, shared state)
        def qk_units(p, qt_nxt, kt_nxt, wq_nxt, wk_nxt):
            units = []
            for w_sb, dest in ((wk_nxt, kt_nxt), (wq_nxt, qt_nxt)):
                for t4 in range(NQ):
                    state = {}

                    def mk(w_sb=w_sb, dest=dest, t4=t4, dc_lo=0, dc_hi=4,
                           state=state):
                        def emit():
                            if dc_lo == 0:
                                state["ps"] = psM.tile([128, 512], f32,
                                                       tag="mm", name="psf")
                            proj_mms(state["ps"], w_sb, t4, dc_lo, dc_hi)
                            if dc_hi == ND:
                                nc.vector.tensor_copy(
                                    out=dest[:, t4 * 512:(t4 + 1) * 512],
                                    in_=state["ps"])
                        return emit

                    units.append(mk(dc_lo=0, dc_hi=4))
                    units.append(mk(dc_lo=4, dc_hi=ND))
            # reorder: t4-major interleaved k/q so early chunks finish first
            order = []
            for t4 in range(NQ):
                for w in range(2):
                    order.append(units[w * 8 + 2 * t4])
                    order.append(units[w * 8 + 2 * t4 + 1])
            return order

        # ---- Phase B: pairs 0-2, next-pair Q/K spread as filler ------------
        qt_cur, kt_cur = qt0, kt0
        deferred = []
        for p in range(NPAIR - 1):
            qt_nxt = qkp.tile([128, T], bf16, tag="qt", name=f"qt{p+1}")
            kt_nxt = qkp.tile([128, T], bf16, tag="kt", name=f"kt{p+1}")
            wq_nxt = dma_w(wq_d, p + 1, "q")
            wk_nxt = dma_w(wk_d, p + 1, "k")
            units = qk_units(p, qt_nxt, kt_nxt, wq_nxt, wk_nxt)
            if p == NPAIR - 2:
                # defer pair-3's t4=3 halves into pair-3's j=0 slots
                deferred = units[12:16]
                units = units[:12]
            spread = Spread(units, 40)
            for hh in range(2):
                for j in range(NQ):
                    attn_group(p, hh, j, qt_cur, kt_cur, spread)
            spread.drain()
            qt_cur, kt_cur = qt_nxt, kt_nxt

        # ---- Pair 3: j-major attention + PSUM-accumulated out-projection --
        def oproj_unit(dc, qc):
            def emit():
                py = psM.tile([128, 512], f32, tag="mm", name="pyo")
                for pp in range(NPAIR):
                    nc.tensor.matmul(
                        py,
                        wo_sb[:, pp, dc, :],
                        otn[:, pp, qc * 512:(qc + 1) * 512],
                        start=(pp == 0),
                        stop=(pp == NPAIR - 1),
                    )
                yt_sb = pyt.tile([128, 512], f32, tag="yt", name="yt_f")
                nc.vector.tensor_copy(out=yt_sb, in_=py)
                nc.sync.dma_start(
                    out=yt_d[dc * 128:(dc + 1) * 128,
                             qc * 512:(qc + 1) * 512],
                    in_=yt_sb,
                )
            return emit

        for j in range(NQ):
            slots = 4 * (j + 1)  # both heads' filler slots at this j
            if j == 0:
                units = deferred
            else:
                units = [oproj_unit(dc, j - 1) for dc in range(ND)]
            spread = Spread(units, slots, start=1 if j else 0)
            attn_group(3, 0, j, qt_cur, kt_cur, spread)
            attn_group(3, 1, j, qt_cur, kt_cur, spread)
            spread.drain()
        for dc in range(ND):
            oproj_unit(dc, NQ - 1)()

    nc.compile()
    return nc


def _pack_inputs(x, Wq, Wk, Wv, Wo):
    """Per-core input maps. Core c: batch c//2, head group c%2."""
    import ml_dtypes

    tri = np.triu(np.ones((128, 128), np.float32)).astype(ml_dtypes.bfloat16)

    def pack_w(W, g):
        # [NPAIR, 128(d_local), ND, 128(e2)]
        out = np.empty((NPAIR, 128, ND, 128), np.float32)
        for p in range(NPAIR):
            h1 = 8 * g + 2 * p
            r = W[[h1, h1 + 1]].transpose(1, 0, 2).reshape(D, 128)  # [d, e2]
            out[p] = r.reshape(ND, 128, 128).transpose(1, 0, 2)
        return np.ascontiguousarray(out).astype(ml_dtypes.bfloat16)

    def pack_wo(Wo, g):
        # [128(e2), NPAIR, ND, 128(d)]
        out = np.empty((128, NPAIR, ND, 128), np.float32)
        for p in range(NPAIR):
            r0 = (8 * g + 2 * p) * 64
            out[:, p] = Wo[r0:r0 + 128].reshape(128, ND, 128)
        return np.ascontiguousarray(out).astype(ml_dtypes.bfloat16)

    packs = {}
    for g in range(2):
        packs[g] = dict(
            wq=pack_w(Wq, g), wk=pack_w(Wk, g), wv=pack_w(Wv, g),
            wo=pack_wo(Wo, g),
        )
    in_maps = []
    for c in range(NCORES):
        b, g = c // 2, c % 2
        m = dict(packs[g])
        xt = x[b].reshape(NQ, 512, ND, 128).transpose(3, 0, 2, 1)
        m["x"] = np.ascontiguousarray(xt).astype(ml_dtypes.bfloat16)
        m["tri"] = tri
        in_maps.append(m)
    return in_maps


def kernel(x, Wq, Wk, Wv, Wo, bo):
    from concourse.bass_utils import run_bass_kernel_spmd

    x = np.asarray(x, np.float32)
    Wq, Wk, Wv = (np.asarray(a, np.float32) for a in (Wq, Wk, Wv))
    Wo = np.asarray(Wo, np.float32)
    bo = np.asarray(bo, np.float32)

    if "nc" not in _CACHE:
        _CACHE["nc"] = _build_program()
    nc = _CACHE["nc"]

    in_maps = _pack_inputs(x, Wq, Wk, Wv, Wo)
    res = run_bass_kernel_spmd(nc, in_maps, list(range(NCORES)))
    _CACHE["last_result"] = res

    out = np.empty((B, T, D), np.float32)
    for b in range(B):
        yt = res.results[2 * b]["yt"] + res.results[2 * b + 1]["yt"]
        out[b] = yt.T + bo
    return out
